# revision 3
# baseline (speedup 1.0000x reference)
"""Expert-parallel MoE (top-1 routing) kernel for 8 TRN2 NeuronCores.

Strategy (per the expert-parallel sharding hint): the 8 experts are sharded
1:1 across the 8 cores. The router is a 0.1%-of-FLOPs linear; it is computed
host-side in float64 to decide the token->expert dispatch (the all-to-all is
realized as the host->device sharding itself: each token's activations are
DMA'd only to the core owning its expert). Each core then runs the dense
expert MLP  y = (silu(x @ gw.T) * (x @ up.T)) @ dw.T  over its gathered
tokens (padded to a uniform capacity C) with fp32 PSUM accumulation.

Layout: everything on device is kept "activation-transposed" so all three
matmuls contract over the partition dimension with zero on-device transposes:
  g_T[i_tile] = sum_k gwT[k, i].T @ x_T[k]      (psum [128(I), C])
  a_T = silu(g_T) * u_T                          (sbuf bf16)
  y_T[m_tile] += dwT[i, m].T @ a_T[i]            (psum [128(H), C], 22-step acc)

Precision: gate/up weights are stored as fp8-e3m4 (power-of-two pre-scale,
descale folded exactly into the silu scale and the DVE multiply), halving
their HBM traffic; down weights and activations stay bf16. The PE runs
mixed-dtype matmuls (fp8 stationary, bf16 moving) at the bf16 rate.

Schedule (v3, from trace analysis of v1/v2):
  * The PE HAM clock-gate keeps the PE at 1.2 GHz until it has been
    continuously busy for ~3.4us, so garbage warm-up matmuls run from
    body start while the first bytes land.
  * DMA triggers (DMA_DIRECT2D) cost ~610ns each on the issuing engine
    and each completion costs ~1-1.5us extra latency in the early
    (shallow-queue) regime, so ALL per-core bytes (x + gate/up fp8 +
    down bf16) are packed host-side into ONE uint8 DRAM blob laid out in
    exact PE-consumption order and streamed as 9 ramping chunks
    (0.27MB..4.2MB) on the single SP HWDGE queue with ONE counting
    semaphore. Matmul tiles read the SBUF blob through bitcast views.
  * y tail: DVE copies psum m0-3, ACT copies m4-7; four small y DMAs
    (m0-1/m2-3 on the SP queue, m4-5/m6-7 on the ACT queue) chase the
    copies so the final transfer is off the critical path ASAP.

Engine streams:
  SP  : 9 blob chunks in consumption order, then y DMA m0-3
  PE  : warm-up MMs; per i: 8 g-MMs, 8 u-MMs, then 8 y-MMs of i-Y_LAG
  ACT : per i: silu(g)->sbuf (with 1/Sg descale); tail: psum->sbuf copies
        m4-7 + y DMA m4-7 on ACT's own HWDGE queue
  DVE : per i: a_T[i] = silu_g * u' * (1/Su) (bf16); tail: copies m0-3
"""

import math

import numpy as np
import ml_dtypes
from contextlib import ExitStack

import concourse.bass as bass
import concourse.mybir as mybir
from concourse.alu_op_type import AluOpType
from concourse.bass_utils import run_bass_kernel_spmd

S, B, H, I, E = 512, 2, 1024, 2816, 8
KT, IT, MT = H // 128, I // 128, H // 128  # 8, 22, 8
_BF = mybir.dt.bfloat16
_F8 = mybir.dt.float8e3  # e3m4
_F32 = mybir.dt.float32
_U8 = mybir.dt.uint8

# CoreSim-only: gate the PE warm-up matmuls on a memset of their input so
# the simulator's uninitialized-read checker stays quiet. On hardware the
# warm-up reads garbage SBUF on purpose (results are discarded), and waiting
# would delay the clock ramp.
SIM_WARMUP_WAIT = False

Y_LAG = 2  # how many i-tiles the down-projection matmuls trail gate/up
WARMUP = 6  # N=512 garbage matmuls at body start: PE busy while the first
#             chunk lands, so the HAM clock-gate un-throttles ~3.4us in
NO_GPSIMD_DRAIN = True  # skip the idle GpSimd engine's costly exit drain


def _blob_layout(C: int):
    """Byte offsets (per partition) of every tile in the input blob,
    in exact PE-consumption order:
      xA (x k0-3) | gate i0 | xB (k4-7) | up i0 | w8 i1 |
      then pairs (w8 i, wd i-2) for i=2..21 | wd 20 | wd 21
    """
    xa = 0
    g0 = xa + 4 * C * 2
    xb = g0 + KT * 128
    u0 = xb + 4 * C * 2
    w8 = [None] * IT
    wd = [None] * IT
    pos = u0 + KT * 128
    w8[0] = None  # i0 is split into g0/u0
    w8[1] = pos
    pos += 2 * KT * 128
    for i in range(2, IT):
        w8[i] = pos
        pos += 2 * KT * 128
        wd[i - 2] = pos
        pos += 2 * MT * 128
    wd[IT - 2] = pos
    pos += 2 * MT * 128
    wd[IT - 1] = pos
    pos += 2 * MT * 128
    return xa, g0, xb, u0, w8, wd, pos


def _chunks(C: int):
    """SP-stream chunk boundaries (bytes) + per-tile landed thresholds."""
    xa, g0, xb, u0, w8, wd, totb = _blob_layout(C)
    cuts = [
        xb,          # c0: xA + gate i0
        w8[1],       # c1: xB + up i0
        w8[2],       # c2: w8 i1
        w8[3],       # c3: w8 i2 + wd0
        w8[4],       # c4: w8 i3 + wd1
        w8[6],       # c5: i4,i5 pairs (wd2,3)
        w8[10],      # c6: i6..i9 pairs (wd4..7)
        w8[15],      # c7: i10..i14 pairs (wd8..12)
        totb,        # c8: i15..i21 + wd13..21
    ]
    bounds = [(0 if n == 0 else cuts[n - 1], cuts[n]) for n in range(len(cuts))]

    def thr_of(off):
        for n, (lo, hi) in enumerate(bounds):
            if lo <= off < hi:
                return 16 * (n + 1)
        raise AssertionError(off)

    w8_thr = [thr_of(g0)] + [thr_of(w8[i]) for i in range(1, IT)]
    wd_thr = [thr_of(wd[i]) for i in range(IT)]
    xb_thr = thr_of(xb)
    # consumption order must see monotone thresholds (single running wait)
    seq = [w8_thr[0], xb_thr]
    for i in range(1, IT):
        seq.append(w8_thr[i])
        if i >= Y_LAG:
            seq.append(wd_thr[i - Y_LAG])
    seq += [wd_thr[IT - 2], wd_thr[IT - 1]]
    assert all(a <= b for a, b in zip(seq, seq[1:])), seq
    return bounds, w8_thr, wd_thr, xb_thr, totb


_nc_cache: dict = {}


def _build(C: int, inv_sg: float, inv_su: float) -> bass.Bass:
    """One-core program; SPMD across 8 cores (same shapes, per-core data)."""
    nc = bass.Bass()
    xa_off, g0_off, xb_off, u0_off, w8_off, wd_off, TOTB = _blob_layout(C)
    bounds, w8_thr, wd_thr, xb_thr, totb = _chunks(C)
    assert totb == TOTB

    src = nc.dram_tensor("src", [128, TOTB], _U8, kind="ExternalInput")
    yt = nc.dram_tensor("yt", [128, MT * C], _BF, kind="ExternalOutput")

    assert C + 256 <= 512, "two y slices must fit one PSUM bank"

    with ExitStack() as ctx:
        blob = ctx.enter_context(nc.sbuf_tensor([128, TOTB], _U8))
        sg_sb = ctx.enter_context(nc.sbuf_tensor([128, IT * C], _F32))
        a_sb = ctx.enter_context(nc.sbuf_tensor([128, IT * C], _BF))
        # y writeback in bf16: halves the tail DMA and doubles copy rate
        # (costs ~0.2% extra output quantization, well inside the budget)
        y_sb = ctx.enter_context(nc.sbuf_tensor([128, MT * C], _BF))
        # every PSUM tensor is one full 2 KiB bank ([128, 512] f32): matmul
        # outputs must not cross bank boundaries, and the bump allocator
        # would otherwise pack tensors across banks
        g_ps = [
            ctx.enter_context(nc.psum_tensor(f"g_ps{j}", [128, 512], _F32))
            for j in range(2)
        ]
        u_ps = [
            ctx.enter_context(nc.psum_tensor(f"u_ps{j}", [128, 512], _F32))
            for j in range(2)
        ]
        y_ps = [
            ctx.enter_context(nc.psum_tensor(f"y_ps{j}", [128, 512], _F32))
            for j in range(4)
        ]

        def yslice(m):
            return y_ps[m // 2][:, (m % 2) * 256 : (m % 2) * 256 + C]

        def x_tile(k):
            off = (xa_off + k * 2 * C) if k < 4 else (xb_off + (k - 4) * 2 * C)
            return blob[:, off : off + 2 * C].bitcast(_BF)

        def gw_tile(i, k):
            base = g0_off if i == 0 else w8_off[i]
            return blob[:, base + k * 128 : base + (k + 1) * 128].bitcast(_F8)

        def uw_tile(i, k):
            base = u0_off if i == 0 else w8_off[i] + KT * 128
            return blob[:, base + k * 128 : base + (k + 1) * 128].bitcast(_F8)

        def dw_tile(i, m):
            base = wd_off[i]
            return blob[:, base + m * 256 : base + (m + 1) * 256].bitcast(_BF)

        warm_sb = ctx.enter_context(nc.sbuf_tensor([128, 512], _BF))

        q_sem = ctx.enter_context(nc.semaphore(name="q_sem"))
        ydma_sem = ctx.enter_context(nc.semaphore(name="ydma_sem"))
        warm_sem = ctx.enter_context(nc.semaphore(name="warm_sem"))
        pe_g = ctx.enter_context(nc.semaphore())
        pe_u = ctx.enter_context(nc.semaphore())
        pe_done = ctx.enter_context(nc.semaphore())
        act_sem = ctx.enter_context(nc.semaphore())
        dve_sem = ctx.enter_context(nc.semaphore())

        block = ctx.enter_context(nc.Block(no_gpsimd_drain=NO_GPSIMD_DRAIN))

        @block.sync
        def _(sync):
            # one SP HWDGE queue, all bytes in exact PE-consumption order,
            # one counting semaphore (in-order queue -> cumulative
            # thresholds). Chunks ramp 0.27MB -> 4.2MB: small first chunks
            # bound the first-matmul latency, large later chunks avoid the
            # ~1us per-instruction completion overhead of the early regime.
            for lo, hi in bounds:
                nc.sync.dma_start(blob[:, lo:hi], src[:, lo:hi]).then_inc(
                    q_sem, 16
                )
            # y writeback m0-3 on the SP queue, chasing the DVE copies
            # (dve_sem is incremented in program order by the DVE alone, so
            # >= IT+2 deterministically means copies m0-1 are done)
            nc.sync.wait_ge(dve_sem, IT + 2)
            nc.sync.dma_start(yt[:, : 2 * C], y_sb[:, : 2 * C]).then_inc(
                ydma_sem, 16
            )
            nc.sync.wait_ge(dve_sem, IT + 4)
            nc.sync.dma_start(
                yt[:, 2 * C : 4 * C], y_sb[:, 2 * C : 4 * C]
            ).then_inc(ydma_sem, 16)
            nc.sync.wait_ge(ydma_sem, 64)

        def y_block(i, stop, inc_each=False):
            for m in range(MT):
                # start=True clears has_written for the WHOLE psum bank,
                # so only the first (even) slice of each bank may set it;
                # the odd slice's first write then lands on cleared
                # has_written and overwrites cleanly.
                mm = nc.tensor.matmul(
                    yslice(m),
                    dw_tile(i, m),
                    a_sb[:, i * C : (i + 1) * C],
                    start=(i == 0 and m % 2 == 0),
                    stop=stop,
                    skip_group_check=True,
                )
                if inc_each:
                    mm.then_inc(pe_done, 1)

        @block.tensor
        def _(tensor):
            # warm-up: garbage matmuls keep the PE busy from body start so
            # the HAM clock-gate un-throttles (1.2 -> 2.4 GHz) ~3.4us in,
            # while the first blob chunk streams. psum bank 0 is
            # re-initialized (start=True) by the first real matmul.
            if WARMUP:
                if SIM_WARMUP_WAIT:
                    nc.tensor.wait_ge(warm_sem, 1)
                for _ in range(WARMUP):
                    nc.tensor.matmul(
                        g_ps[0][:],
                        warm_sb[:, :128],
                        warm_sb[:],
                        start=True,
                        stop=True,
                    )
            cur_thr = 0

            def q_wait(thr):
                nonlocal cur_thr
                if thr > cur_thr:
                    nc.tensor.wait_ge(q_sem, thr)
                    cur_thr = thr

            for i in range(IT):
                pp = i % 2
                q_wait(w8_thr[i])
                if i >= 2:
                    # covers g/u psum bank reuse (mul(i-2) drained) and,
                    # for Y_LAG==2, a_T[i-2] readiness for the y-block
                    nc.tensor.wait_ge(dve_sem, i - 1)
                for k in range(KT):
                    if i == 0 and k == 4:
                        q_wait(xb_thr)
                    mm = nc.tensor.matmul(
                        g_ps[pp][:, :C],
                        gw_tile(i, k),
                        x_tile(k),
                        start=(k == 0),
                        stop=(k == KT - 1),
                    )
                mm.then_inc(pe_g, 1)
                for k in range(KT):
                    mm = nc.tensor.matmul(
                        u_ps[pp][:, :C],
                        uw_tile(i, k),
                        x_tile(k),
                        start=(k == 0),
                        stop=(k == KT - 1),
                    )
                mm.then_inc(pe_u, 1)
                if i >= Y_LAG:
                    # y-matmuls trail gate/up by Y_LAG i-tiles so ACT->DVE
                    # chain latency never stalls the PE
                    iy = i - Y_LAG
                    if Y_LAG == 1:
                        nc.tensor.wait_ge(dve_sem, i)
                    q_wait(wd_thr[iy])
                    y_block(iy, stop=False)
            for iy in range(IT - Y_LAG, IT - 1):
                nc.tensor.wait_ge(dve_sem, iy + 1)
                q_wait(wd_thr[iy])
                y_block(iy, stop=False)
            nc.tensor.wait_ge(dve_sem, IT)
            q_wait(wd_thr[IT - 1])
            y_block(IT - 1, stop=True, inc_each=True)

        @block.scalar
        def _(scalar):
            for i in range(IT):
                pp = i % 2
                nc.scalar.wait_ge(pe_g, i + 1)
                nc.scalar.activation(
                    sg_sb[:, i * C : (i + 1) * C],
                    g_ps[pp][:, :C],
                    mybir.ActivationFunctionType.Silu,
                    scale=inv_sg,
                ).then_inc(act_sem, 1)
            # tail: psum->sbuf copies m4-7 + y DMA on ACT's own HWDGE queue
            for m in range(4, MT):
                nc.scalar.wait_ge(pe_done, m + 1)
                nc.scalar.copy(y_sb[:, m * C : (m + 1) * C], yslice(m))
                if m in (5, 7):
                    # no explicit wait: copies precede the DMA in ACT's
                    # in-order stream, and HWDGE descriptor generation
                    # happens at instruction execution time
                    nc.scalar.dma_start(
                        yt[:, (m - 1) * C : (m + 1) * C],
                        y_sb[:, (m - 1) * C : (m + 1) * C],
                    ).then_inc(ydma_sem, 16)

        @block.vector
        def _(vector):
            if SIM_WARMUP_WAIT:
                nc.vector.memset(warm_sb[:], 0.0).then_inc(warm_sem, 1)
            for i in range(IT):
                pp = i % 2
                nc.vector.wait_ge(act_sem, i + 1)
                nc.vector.wait_ge(pe_u, i + 1)
                # a = (u' * 1/Su) * silu_g   (1/Su is a power of two)
                nc.vector.scalar_tensor_tensor(
                    a_sb[:, i * C : (i + 1) * C],
                    u_ps[pp][:, :C],
                    inv_su,
                    sg_sb[:, i * C : (i + 1) * C],
                    AluOpType.mult,
                    AluOpType.mult,
                ).then_inc(dve_sem, 1)
            for m in range(4):
                nc.vector.wait_ge(pe_done, m + 1)
                nc.vector.tensor_copy(
                    y_sb[:, m * C : (m + 1) * C], yslice(m)
                ).then_inc(dve_sem, 1)

    return nc


def _bf(x):
    return np.ascontiguousarray(x).astype(ml_dtypes.bfloat16)


def _pow2_scale(absmax: float, dt) -> float:
    fmax = float(ml_dtypes.finfo(dt).max)
    return 2.0 ** math.floor(math.log2((fmax * 0.5) / absmax))


def run(hidden_states, router_w, gate_w, up_w, down_w, trace=False):
    h = np.asarray(hidden_states, dtype=np.float32)
    rw = np.asarray(router_w, dtype=np.float32)
    gw = np.asarray(gate_w, dtype=np.float32)
    uw = np.asarray(up_w, dtype=np.float32)
    dw = np.asarray(down_w, dtype=np.float32)

    T = S * B
    hf = h.reshape(T, H)
    logits = hf.astype(np.float64) @ rw.astype(np.float64).T
    ids = logits.argmax(-1)
    idx = [np.where(ids == e)[0] for e in range(E)]
    maxc = max(len(s) for s in idx)
    C = max(128, -(-maxc // 4) * 4)

    sg = _pow2_scale(float(np.abs(gw).max()), ml_dtypes.float8_e3m4)
    su = _pow2_scale(float(np.abs(uw).max()), ml_dtypes.float8_e3m4)

    key = (C, sg, su, Y_LAG, WARMUP, NO_GPSIMD_DRAIN)
    if key not in _nc_cache:
        _nc_cache[key] = _build(C, 1.0 / sg, 1.0 / su)
    nc = _nc_cache[key]

    xa_off, g0_off, xb_off, u0_off, w8_off, wd_off, TOTB = _blob_layout(C)

    in_maps = []
    for e in range(E):
        sel = idx[e]
        xp = np.zeros((C, H), np.float32)
        xp[: len(sel)] = hf[sel]
        # xt[p, k*C+c] = x[c, k*128+p]
        xt = _bf(xp.reshape(C, KT, 128).transpose(2, 1, 0).reshape(128, KT * C))
        # gwt[i, p, k*128+m] = gate_w[e][i*128+m, k*128+p]
        gwt = gw[e].reshape(IT, 128, KT, 128).transpose(0, 3, 2, 1).reshape(IT, 128, KT * 128)
        uwt = uw[e].reshape(IT, 128, KT, 128).transpose(0, 3, 2, 1).reshape(IT, 128, KT * 128)
        # dwt[i, p, m*128+mm] = down_w[e][m*128+mm, i*128+p]
        dwt = dw[e].reshape(MT, 128, IT, 128).transpose(2, 3, 0, 1).reshape(IT, 128, MT * 128)
        g8 = np.ascontiguousarray((gwt * sg).transpose(1, 0, 2)).astype(
            ml_dtypes.float8_e3m4
        )  # [128, IT, 1024]
        u8 = np.ascontiguousarray((uwt * su).transpose(1, 0, 2)).astype(
            ml_dtypes.float8_e3m4
        )
        wdb = _bf(dwt.transpose(1, 0, 2))  # [128, IT, 1024] bf16

        xbytes = xt.view(np.uint8)  # [128, KT*C*2]
        g8b = g8.view(np.uint8)  # [128, IT, 1024]
        u8b = u8.view(np.uint8)
        wdbb = wdb.view(np.uint8).reshape(128, IT, 2 * MT * 128)

        blob = np.empty((128, TOTB), np.uint8)
        blob[:, xa_off : xa_off + 8 * C] = xbytes[:, : 8 * C]
        blob[:, g0_off : g0_off + 1024] = g8b[:, 0]
        blob[:, xb_off : xb_off + 8 * C] = xbytes[:, 8 * C :]
        blob[:, u0_off : u0_off + 1024] = u8b[:, 0]
        for i in range(1, IT):
            o = w8_off[i]
            blob[:, o : o + 1024] = g8b[:, i]
            blob[:, o + 1024 : o + 2048] = u8b[:, i]
        for i in range(IT):
            o = wd_off[i]
            blob[:, o : o + 2048] = wdbb[:, i]
        in_maps.append({"src": blob})

    res = run_bass_kernel_spmd(nc, in_maps, core_ids=list(range(E)), trace=trace)

    out = np.zeros((T, H), np.float32)
    for e in range(E):
        ytv = np.asarray(res.results[e]["yt"]).astype(np.float32)
        # y[c, m*128+p] = yt[p, m*C+c]
        y = ytv.reshape(128, MT, C).transpose(2, 1, 0).reshape(C, H)
        out[idx[e]] = y[: len(idx[e])]
    return out.reshape(S, B, H), res


def kernel(**inputs) -> np.ndarray:
    out, _ = run(**inputs)
    return out


# revision 4
# speedup vs baseline: 1.1939x; 1.1939x over previous
"""Expert-parallel MoE (top-1 routing) kernel for 8 TRN2 NeuronCores.

Strategy (per the expert-parallel sharding hint): the 8 experts are sharded
1:1 across the 8 cores. The router is a 0.1%-of-FLOPs linear; it is computed
host-side in float64 to decide the token->expert dispatch (the all-to-all is
realized as the host->device sharding itself: each token's activations are
DMA'd only to the core owning its expert). Each core then runs the dense
expert MLP  y = (silu(x @ gw.T) * (x @ up.T)) @ dw.T  over its gathered
tokens (padded to a uniform capacity C) with fp32 PSUM accumulation.

Layout: everything on device is kept "activation-transposed" so all three
matmuls contract over the partition dimension with zero on-device transposes:
  g_T[i_tile] = sum_k gwT[k, i].T @ x_T[k]      (psum [128(I), C])
  a_T = silu(g_T) * u_T                          (sbuf bf16)
  y_T[m_tile] += dwT[i, m].T @ a_T[i]            (psum [128(H), C], 22-step acc)

Precision: gate/up weights are stored as fp8-e3m4 (power-of-two pre-scale,
descale folded exactly into the silu scale and the DVE multiply), halving
their HBM traffic; down weights and activations stay bf16. The PE runs
mixed-dtype matmuls (fp8 stationary, bf16 moving) at the bf16 rate.

Schedule (v3, from trace analysis of v1/v2):
  * The PE HAM clock-gate keeps the PE at 1.2 GHz until it has been
    continuously busy for ~3.4us, so garbage warm-up matmuls run from
    body start while the first bytes land.
  * DMA triggers (DMA_DIRECT2D) cost ~610ns each on the issuing engine
    and each completion costs ~1-1.5us extra latency in the early
    (shallow-queue) regime, so ALL per-core bytes (x + gate/up fp8 +
    down bf16) are packed host-side into ONE uint8 DRAM blob laid out in
    exact PE-consumption order and streamed as 9 ramping chunks
    (0.27MB..4.2MB) on the single SP HWDGE queue with ONE counting
    semaphore. Matmul tiles read the SBUF blob through bitcast views.
  * y tail: DVE copies psum m0-3, ACT copies m4-7; four small y DMAs
    (m0-1/m2-3 on the SP queue, m4-5/m6-7 on the ACT queue) chase the
    copies so the final transfer is off the critical path ASAP.

Engine streams:
  SP  : 9 blob chunks in consumption order, then y DMA m0-3
  PE  : warm-up MMs; per i: 8 g-MMs, 8 u-MMs, then 8 y-MMs of i-Y_LAG
  ACT : per i: silu(g)->sbuf (with 1/Sg descale); tail: psum->sbuf copies
        m4-7 + y DMA m4-7 on ACT's own HWDGE queue
  DVE : per i: a_T[i] = silu_g * u' * (1/Su) (bf16); tail: copies m0-3
"""

import math

import numpy as np
import ml_dtypes
from contextlib import ExitStack

import concourse.bass as bass
import concourse.mybir as mybir
from concourse.alu_op_type import AluOpType
from concourse.bass_utils import run_bass_kernel_spmd

S, B, H, I, E = 512, 2, 1024, 2816, 8
KT, IT, MT = H // 128, I // 128, H // 128  # 8, 22, 8
_BF = mybir.dt.bfloat16
_F8 = mybir.dt.float8e3  # e3m4
_F32 = mybir.dt.float32
_U8 = mybir.dt.uint8

# CoreSim-only: gate the PE warm-up matmuls on a memset of their input so
# the simulator's uninitialized-read checker stays quiet. On hardware the
# warm-up reads garbage SBUF on purpose (results are discarded), and waiting
# would delay the clock ramp.
SIM_WARMUP_WAIT = False

Y_LAG = 2  # how many i-tiles the down-projection matmuls trail gate/up
WARMUP = 6  # N=512 garbage matmuls at body start: PE busy while the first
#             chunk lands, so the HAM clock-gate un-throttles ~3.4us in
NO_GPSIMD_DRAIN = True  # skip the idle GpSimd engine's costly exit drain


def _blob_layout(C: int):
    """Byte offsets (per partition) of every tile in the input blob,
    in exact PE-consumption order:
      xA (x k0-3) | gate i0 | xB (k4-7) | up i0 | w8 i1 |
      then pairs (w8 i, wd i-2) for i=2..21 | wd 20 | wd 21
    """
    xa = 0
    g0 = xa + 4 * C * 2
    xb = g0 + KT * 128
    u0 = xb + 4 * C * 2
    w8 = [None] * IT
    wd = [None] * IT
    pos = u0 + KT * 128
    w8[0] = None  # i0 is split into g0/u0
    w8[1] = pos
    pos += 2 * KT * 128
    for i in range(2, IT):
        w8[i] = pos
        pos += 2 * KT * 128
        wd[i - 2] = pos
        pos += 2 * MT * 128
    wd[IT - 2] = pos
    pos += 2 * MT * 128
    wd[IT - 1] = pos
    pos += 2 * MT * 128
    return xa, g0, xb, u0, w8, wd, pos


def _chunks(C: int):
    """SP-stream chunk boundaries (bytes) + per-tile landed thresholds."""
    xa, g0, xb, u0, w8, wd, totb = _blob_layout(C)
    # Completion granularity must track PE consumption (a chunk's landed-
    # semaphore only fires once the WHOLE chunk is in SBUF), so chunks stay
    # <= 2 i-tile pairs (~1MB): the sem lags the data by <=2.5us, less than
    # the PE's consumption slack once the stream is one chunk ahead.
    cuts = [
        xb,          # c0: xA + gate i0
        w8[1],       # c1: xB + up i0
        w8[2],       # c2: w8 i1
        w8[3],       # c3: w8 i2 + wd0
        w8[4],       # c4: w8 i3 + wd1
        w8[5],       # c5: w8 i4 + wd2
        w8[6],       # c6: w8 i5 + wd3
        w8[8],       # c7: i6,i7 pairs
        w8[10],      # c8: i8,i9
        w8[12],      # c9: i10,i11
        w8[14],      # c10: i12,i13
        w8[16],      # c11: i14,i15
        w8[18],      # c12: i16,i17
        w8[20],      # c13: i18,i19
        wd[20],      # c14: i20,i21 + wd18,19
        totb,        # c15: wd20,21
    ]
    bounds = [(0 if n == 0 else cuts[n - 1], cuts[n]) for n in range(len(cuts))]

    def thr_of(off):
        for n, (lo, hi) in enumerate(bounds):
            if lo <= off < hi:
                return 16 * (n + 1)
        raise AssertionError(off)

    w8_thr = [thr_of(g0)] + [thr_of(w8[i]) for i in range(1, IT)]
    wd_thr = [thr_of(wd[i]) for i in range(IT)]
    xb_thr = thr_of(xb)
    # consumption order must see monotone thresholds (single running wait)
    seq = [w8_thr[0], xb_thr]
    for i in range(1, IT):
        seq.append(w8_thr[i])
        if i >= Y_LAG:
            seq.append(wd_thr[i - Y_LAG])
    seq += [wd_thr[IT - 2], wd_thr[IT - 1]]
    assert all(a <= b for a, b in zip(seq, seq[1:])), seq
    return bounds, w8_thr, wd_thr, xb_thr, totb


_nc_cache: dict = {}


def _build(C: int, inv_sg: float, inv_su: float) -> bass.Bass:
    """One-core program; SPMD across 8 cores (same shapes, per-core data)."""
    nc = bass.Bass()
    xa_off, g0_off, xb_off, u0_off, w8_off, wd_off, TOTB = _blob_layout(C)
    bounds, w8_thr, wd_thr, xb_thr, totb = _chunks(C)
    assert totb == TOTB

    src = nc.dram_tensor("src", [128, TOTB], _U8, kind="ExternalInput")
    yt = nc.dram_tensor("yt", [128, MT * C], _BF, kind="ExternalOutput")

    assert C + 256 <= 512, "two y slices must fit one PSUM bank"

    with ExitStack() as ctx:
        blob = ctx.enter_context(nc.sbuf_tensor([128, TOTB], _U8))
        sg_sb = ctx.enter_context(nc.sbuf_tensor([128, IT * C], _F32))
        a_sb = ctx.enter_context(nc.sbuf_tensor([128, IT * C], _BF))
        # y writeback in bf16: halves the tail DMA and doubles copy rate
        # (costs ~0.2% extra output quantization, well inside the budget)
        y_sb = ctx.enter_context(nc.sbuf_tensor([128, MT * C], _BF))
        # every PSUM tensor is one full 2 KiB bank ([128, 512] f32): matmul
        # outputs must not cross bank boundaries, and the bump allocator
        # would otherwise pack tensors across banks
        g_ps = [
            ctx.enter_context(nc.psum_tensor(f"g_ps{j}", [128, 512], _F32))
            for j in range(2)
        ]
        u_ps = [
            ctx.enter_context(nc.psum_tensor(f"u_ps{j}", [128, 512], _F32))
            for j in range(2)
        ]
        y_ps = [
            ctx.enter_context(nc.psum_tensor(f"y_ps{j}", [128, 512], _F32))
            for j in range(4)
        ]

        def yslice(m):
            return y_ps[m // 2][:, (m % 2) * 256 : (m % 2) * 256 + C]

        def x_tile(k):
            off = (xa_off + k * 2 * C) if k < 4 else (xb_off + (k - 4) * 2 * C)
            return blob[:, off : off + 2 * C].bitcast(_BF)

        def gw_tile(i, k):
            base = g0_off if i == 0 else w8_off[i]
            return blob[:, base + k * 128 : base + (k + 1) * 128].bitcast(_F8)

        def uw_tile(i, k):
            base = u0_off if i == 0 else w8_off[i] + KT * 128
            return blob[:, base + k * 128 : base + (k + 1) * 128].bitcast(_F8)

        def dw_tile(i, m):
            base = wd_off[i]
            return blob[:, base + m * 256 : base + (m + 1) * 256].bitcast(_BF)

        warm_sb = ctx.enter_context(nc.sbuf_tensor([128, 512], _BF))

        q_sem = ctx.enter_context(nc.semaphore(name="q_sem"))
        ydma_sem = ctx.enter_context(nc.semaphore(name="ydma_sem"))
        warm_sem = ctx.enter_context(nc.semaphore(name="warm_sem"))
        pe_g = ctx.enter_context(nc.semaphore())
        pe_u = ctx.enter_context(nc.semaphore())
        pe_done = ctx.enter_context(nc.semaphore())
        act_sem = ctx.enter_context(nc.semaphore())
        dve_sem = ctx.enter_context(nc.semaphore())

        block = ctx.enter_context(nc.Block(no_gpsimd_drain=NO_GPSIMD_DRAIN))

        @block.sync
        def _(sync):
            # one SP HWDGE queue, all bytes in exact PE-consumption order,
            # one counting semaphore (in-order queue -> cumulative
            # thresholds). Chunks ramp 0.27MB -> 4.2MB: small first chunks
            # bound the first-matmul latency, large later chunks avoid the
            # ~1us per-instruction completion overhead of the early regime.
            for lo, hi in bounds:
                nc.sync.dma_start(blob[:, lo:hi], src[:, lo:hi]).then_inc(
                    q_sem, 16
                )
            # y writeback m0-3 on the SP queue, chasing the DVE copies
            # (dve_sem is incremented in program order by the DVE alone, so
            # >= IT+2 deterministically means copies m0-1 are done)
            nc.sync.wait_ge(dve_sem, IT + 2)
            nc.sync.dma_start(yt[:, : 2 * C], y_sb[:, : 2 * C]).then_inc(
                ydma_sem, 16
            )
            nc.sync.wait_ge(dve_sem, IT + 4)
            nc.sync.dma_start(
                yt[:, 2 * C : 4 * C], y_sb[:, 2 * C : 4 * C]
            ).then_inc(ydma_sem, 16)
            nc.sync.wait_ge(ydma_sem, 64)

        def y_block(i, stop, inc_each=False):
            for m in range(MT):
                # start=True clears has_written for the WHOLE psum bank,
                # so only the first (even) slice of each bank may set it;
                # the odd slice's first write then lands on cleared
                # has_written and overwrites cleanly.
                mm = nc.tensor.matmul(
                    yslice(m),
                    dw_tile(i, m),
                    a_sb[:, i * C : (i + 1) * C],
                    start=(i == 0 and m % 2 == 0),
                    stop=stop,
                    skip_group_check=True,
                )
                if inc_each:
                    mm.then_inc(pe_done, 1)

        @block.tensor
        def _(tensor):
            # warm-up: garbage matmuls keep the PE busy from body start so
            # the HAM clock-gate un-throttles (1.2 -> 2.4 GHz) ~3.4us in,
            # while the first blob chunk streams. psum bank 0 is
            # re-initialized (start=True) by the first real matmul.
            if WARMUP:
                if SIM_WARMUP_WAIT:
                    nc.tensor.wait_ge(warm_sem, 1)
                for _ in range(WARMUP):
                    nc.tensor.matmul(
                        g_ps[0][:],
                        warm_sb[:, :128],
                        warm_sb[:],
                        start=True,
                        stop=True,
                    )
            cur_thr = 0

            def q_wait(thr):
                nonlocal cur_thr
                if thr > cur_thr:
                    nc.tensor.wait_ge(q_sem, thr)
                    cur_thr = thr

            for i in range(IT):
                pp = i % 2
                q_wait(w8_thr[i])
                if i >= 2:
                    # covers g/u psum bank reuse (mul(i-2) drained) and,
                    # for Y_LAG==2, a_T[i-2] readiness for the y-block
                    nc.tensor.wait_ge(dve_sem, i - 1)
                for k in range(KT):
                    if i == 0 and k == 4:
                        q_wait(xb_thr)
                    mm = nc.tensor.matmul(
                        g_ps[pp][:, :C],
                        gw_tile(i, k),
                        x_tile(k),
                        start=(k == 0),
                        stop=(k == KT - 1),
                    )
                mm.then_inc(pe_g, 1)
                for k in range(KT):
                    mm = nc.tensor.matmul(
                        u_ps[pp][:, :C],
                        uw_tile(i, k),
                        x_tile(k),
                        start=(k == 0),
                        stop=(k == KT - 1),
                    )
                mm.then_inc(pe_u, 1)
                if i >= Y_LAG:
                    # y-matmuls trail gate/up by Y_LAG i-tiles so ACT->DVE
                    # chain latency never stalls the PE
                    iy = i - Y_LAG
                    if Y_LAG == 1:
                        nc.tensor.wait_ge(dve_sem, i)
                    q_wait(wd_thr[iy])
                    y_block(iy, stop=False)
            for iy in range(IT - Y_LAG, IT - 1):
                nc.tensor.wait_ge(dve_sem, iy + 1)
                q_wait(wd_thr[iy])
                y_block(iy, stop=False)
            nc.tensor.wait_ge(dve_sem, IT)
            q_wait(wd_thr[IT - 1])
            y_block(IT - 1, stop=True, inc_each=True)

        @block.scalar
        def _(scalar):
            for i in range(IT):
                pp = i % 2
                nc.scalar.wait_ge(pe_g, i + 1)
                nc.scalar.activation(
                    sg_sb[:, i * C : (i + 1) * C],
                    g_ps[pp][:, :C],
                    mybir.ActivationFunctionType.Silu,
                    scale=inv_sg,
                ).then_inc(act_sem, 1)
            # tail: psum->sbuf copies m4-7 + y DMA on ACT's own HWDGE queue
            for m in range(4, MT):
                nc.scalar.wait_ge(pe_done, m + 1)
                nc.scalar.copy(y_sb[:, m * C : (m + 1) * C], yslice(m))
                if m in (5, 7):
                    # no explicit wait: copies precede the DMA in ACT's
                    # in-order stream, and HWDGE descriptor generation
                    # happens at instruction execution time
                    nc.scalar.dma_start(
                        yt[:, (m - 1) * C : (m + 1) * C],
                        y_sb[:, (m - 1) * C : (m + 1) * C],
                    ).then_inc(ydma_sem, 16)

        @block.vector
        def _(vector):
            if SIM_WARMUP_WAIT:
                nc.vector.memset(warm_sb[:], 0.0).then_inc(warm_sem, 1)
            for i in range(IT):
                pp = i % 2
                nc.vector.wait_ge(act_sem, i + 1)
                nc.vector.wait_ge(pe_u, i + 1)
                # a = (u' * 1/Su) * silu_g   (1/Su is a power of two)
                nc.vector.scalar_tensor_tensor(
                    a_sb[:, i * C : (i + 1) * C],
                    u_ps[pp][:, :C],
                    inv_su,
                    sg_sb[:, i * C : (i + 1) * C],
                    AluOpType.mult,
                    AluOpType.mult,
                ).then_inc(dve_sem, 1)
            for m in range(4):
                nc.vector.wait_ge(pe_done, m + 1)
                nc.vector.tensor_copy(
                    y_sb[:, m * C : (m + 1) * C], yslice(m)
                ).then_inc(dve_sem, 1)

    return nc


def _bf(x):
    return np.ascontiguousarray(x).astype(ml_dtypes.bfloat16)


def _pow2_scale(absmax: float, dt) -> float:
    fmax = float(ml_dtypes.finfo(dt).max)
    return 2.0 ** math.floor(math.log2((fmax * 0.5) / absmax))


def run(hidden_states, router_w, gate_w, up_w, down_w, trace=False):
    h = np.asarray(hidden_states, dtype=np.float32)
    rw = np.asarray(router_w, dtype=np.float32)
    gw = np.asarray(gate_w, dtype=np.float32)
    uw = np.asarray(up_w, dtype=np.float32)
    dw = np.asarray(down_w, dtype=np.float32)

    T = S * B
    hf = h.reshape(T, H)
    logits = hf.astype(np.float64) @ rw.astype(np.float64).T
    ids = logits.argmax(-1)
    idx = [np.where(ids == e)[0] for e in range(E)]
    maxc = max(len(s) for s in idx)
    C = max(128, -(-maxc // 4) * 4)

    sg = _pow2_scale(float(np.abs(gw).max()), ml_dtypes.float8_e3m4)
    su = _pow2_scale(float(np.abs(uw).max()), ml_dtypes.float8_e3m4)

    key = (C, sg, su, Y_LAG, WARMUP, NO_GPSIMD_DRAIN)
    if key not in _nc_cache:
        _nc_cache[key] = _build(C, 1.0 / sg, 1.0 / su)
    nc = _nc_cache[key]

    xa_off, g0_off, xb_off, u0_off, w8_off, wd_off, TOTB = _blob_layout(C)

    in_maps = []
    for e in range(E):
        sel = idx[e]
        xp = np.zeros((C, H), np.float32)
        xp[: len(sel)] = hf[sel]
        # xt[p, k*C+c] = x[c, k*128+p]
        xt = _bf(xp.reshape(C, KT, 128).transpose(2, 1, 0).reshape(128, KT * C))
        # gwt[i, p, k*128+m] = gate_w[e][i*128+m, k*128+p]
        gwt = gw[e].reshape(IT, 128, KT, 128).transpose(0, 3, 2, 1).reshape(IT, 128, KT * 128)
        uwt = uw[e].reshape(IT, 128, KT, 128).transpose(0, 3, 2, 1).reshape(IT, 128, KT * 128)
        # dwt[i, p, m*128+mm] = down_w[e][m*128+mm, i*128+p]
        dwt = dw[e].reshape(MT, 128, IT, 128).transpose(2, 3, 0, 1).reshape(IT, 128, MT * 128)
        g8 = np.ascontiguousarray((gwt * sg).transpose(1, 0, 2)).astype(
            ml_dtypes.float8_e3m4
        )  # [128, IT, 1024]
        u8 = np.ascontiguousarray((uwt * su).transpose(1, 0, 2)).astype(
            ml_dtypes.float8_e3m4
        )
        wdb = _bf(dwt.transpose(1, 0, 2))  # [128, IT, 1024] bf16

        xbytes = xt.view(np.uint8)  # [128, KT*C*2]
        g8b = g8.view(np.uint8)  # [128, IT, 1024]
        u8b = u8.view(np.uint8)
        wdbb = wdb.view(np.uint8).reshape(128, IT, 2 * MT * 128)

        blob = np.empty((128, TOTB), np.uint8)
        blob[:, xa_off : xa_off + 8 * C] = xbytes[:, : 8 * C]
        blob[:, g0_off : g0_off + 1024] = g8b[:, 0]
        blob[:, xb_off : xb_off + 8 * C] = xbytes[:, 8 * C :]
        blob[:, u0_off : u0_off + 1024] = u8b[:, 0]
        for i in range(1, IT):
            o = w8_off[i]
            blob[:, o : o + 1024] = g8b[:, i]
            blob[:, o + 1024 : o + 2048] = u8b[:, i]
        for i in range(IT):
            o = wd_off[i]
            blob[:, o : o + 2048] = wdbb[:, i]
        in_maps.append({"src": blob})

    res = run_bass_kernel_spmd(nc, in_maps, core_ids=list(range(E)), trace=trace)

    out = np.zeros((T, H), np.float32)
    for e in range(E):
        ytv = np.asarray(res.results[e]["yt"]).astype(np.float32)
        # y[c, m*128+p] = yt[p, m*C+c]
        y = ytv.reshape(128, MT, C).transpose(2, 1, 0).reshape(C, H)
        out[idx[e]] = y[: len(idx[e])]
    return out.reshape(S, B, H), res


def kernel(**inputs) -> np.ndarray:
    out, _ = run(**inputs)
    return out
